# revision 37
# baseline (speedup 1.0000x reference)
"""Multi-head attention (B=4, T=2048, dim=2048, H=16, RoPE) on 8 TRN2 NeuronCores.

Tensor-parallel over heads: core c owns heads {2c, 2c+1} (projection dim
slice [256c, 256c+256)).  Each core computes q/k/v projections for its
heads, RoPE, full softmax attention for its 8 (batch, head) pairs, and a
partial output projection against its 256-row slice of wo; the host sums
the 8 bf16 partial outputs and adds wo_b plus the (exact) v-bias term
wv_b @ wo_w.T.

All matmuls run in bf16 with f32 PSUM accumulation; softmax runs exp in
f32->bf16 on the scalar engine with denominators accumulated via an extra
ones-column on V through the PV matmul.

v2 schedule: q/k PSUM is evicted by a single scalar-engine copy (frees the
bank in ~1us) and RoPE runs as six bf16 STT ops on the vector engine in
4x mode; the out-projection for batch b is interleaved into attention(b)
span-by-span (s outer, m inner) so the tail holds only the last span;
partial outputs stream out as bf16; weights load on parallel DMA queues.
"""

import json
import sys

sys.path.insert(0, "/opt/trn_rl_repo")

import ml_dtypes
import numpy as np

BF16 = ml_dtypes.bfloat16

# Problem shape (hardcoded per contract).
B, T, D = 4, 2048, 2048
H = 16
N_CORES = 8
HL = H // N_CORES  # heads per core = 2
DH = D // H  # head dim = 128
DOUT = HL * DH  # per-core projection width = 256
BT = B * T  # 8192 tokens
P = 128
NK = D // P  # 16 feature chunks
SPAN = 512
NSPAN = T // SPAN  # 4 token spans per batch
NTT = T // P  # 16 token tiles per batch


# ---------------------------------------------------------------------------
# BIR legalization: the walrus build in this container rejects instructions
# carrying more than one sync wait. Engines execute their stream in order, so
# hoisting excess waits into standalone EventSemaphore instructions directly
# before the instruction (same engine) is semantically equivalent; Tile's
# dependency graph is acyclic so this cannot deadlock.
# ---------------------------------------------------------------------------


def _legalize_waits(bir_json: bytes, max_inline: int = 1, es_capacity: int = 2):
    bir = json.loads(bir_json)
    for f in bir.get("functions", []):
        for bb in f.get("blocks", []):
            out = []
            for inst in bb.get("instructions", []):
                si = inst.get("sync_info")
                waits = (si or {}).get("on_wait") or []
                cap = (
                    es_capacity
                    if inst.get("opcode") == "EventSemaphore"
                    else max_inline
                )
                if len(waits) > cap:
                    keep, excess = waits[:cap], waits[cap:]
                    for ci in range(0, len(excess), es_capacity):
                        out.append(
                            {
                                "debug": inst.get("debug", 0),
                                "engine": inst["engine"],
                                "ins": [],
                                "name": f"{inst['name']}_xw{ci}",
                                "opcode": "EventSemaphore",
                                "outs": [],
                                "sync_info": {
                                    "on_update": [],
                                    "on_wait": excess[ci : ci + es_capacity],
                                },
                            }
                        )
                    si["on_wait"] = keep
                out.append(inst)
            bb["instructions"] = out
    return json.dumps(bir).encode()


_patched = False


def _install_compile_patch():
    global _patched
    if _patched:
        return
    _patched = True
    from concourse import bass2jax, bass_utils

    orig = bass_utils.compile_bir_kernel

    def patched_compile(bir_json, tmpdir, neff_name="file.neff"):
        return orig(_legalize_waits(bir_json), tmpdir, neff_name)

    bass2jax.compile_bir_kernel = patched_compile


# ---------------------------------------------------------------------------
# Kernel builder (one SPMD graph; per-core behavior differs only via inputs)
# ---------------------------------------------------------------------------


def _build_nc():
    import concourse.bass as bass
    import concourse.tile as tile
    from concourse import mybir
    from concourse.masks import make_identity

    f32 = mybir.dt.float32
    bf16 = mybir.dt.bfloat16

    nc = bass.Bass()
    xT = nc.declare_dram_parameter("xT", [D, BT], bf16, isOutput=False)
    wqT = nc.declare_dram_parameter("wqT", [D + 1, DOUT], bf16, isOutput=False)
    wkT = nc.declare_dram_parameter("wkT", [D + 1, DOUT], bf16, isOutput=False)
    wvT = nc.declare_dram_parameter("wvT", [D, DOUT], bf16, isOutput=False)
    woT = nc.declare_dram_parameter("woT", [DOUT, D], bf16, isOutput=False)
    cosT = nc.declare_dram_parameter("cosT", [DH // 2, T], bf16, isOutput=False)
    sinT = nc.declare_dram_parameter("sinT", [DH // 2, T], bf16, isOutput=False)
    outp = nc.declare_dram_parameter("out", [BT, D], bf16, isOutput=True)

    HDH = DH + 1  # head slot width in v_ones (128 v cols + ones col)
    hh = DH // 2
    Copy = mybir.ActivationFunctionType.Copy
    add = mybir.AluOpType.add
    mult = mybir.AluOpType.mult
    sub = mybir.AluOpType.subtract

    with tile.TileContext(nc) as tc:
        with (
            tc.tile_pool(name="wpool", bufs=1) as wpool,
            tc.tile_pool(name="xpool", bufs=2) as xpool,
            tc.tile_pool(name="qkv", bufs=1) as qkv,
            tc.tile_pool(name="qkT", bufs=2) as qkT,
            tc.tile_pool(name="aop", bufs=1) as aop,
            tc.tile_pool(name="cpool", bufs=1) as cpool,
            tc.tile_pool(name="epool", bufs=2) as epool,
            tc.tile_pool(name="misc", bufs=1) as misc,
            tc.tile_pool(name="recp", bufs=4) as recp,
            tc.tile_pool(name="obuf", bufs=3) as obuf,
            tc.tile_pool(name="psS", bufs=2, space="PSUM") as psS,
            tc.tile_pool(name="ps512", bufs=2, space="PSUM") as ps512,
            tc.tile_pool(name="pspv", bufs=2, space="PSUM") as pspv,
        ):
            # ---- persistent: weights, tables, identity, bias columns ----
            ident = wpool.tile([P, P], bf16, tag="ident")
            make_identity(nc, ident)
            zeroT = wpool.tile([P, 2 * SPAN], bf16, tag="zeroT")
            nc.vector.memset(zeroT, 0.0)

            def load_wT(name, dram, eng, nchunk=4):
                wsb = wpool.tile([P, NK, DOUT], bf16, tag=name)
                wsrc = dram[:D, :].rearrange("(ko p) d -> p ko d", p=P)
                kper = NK // nchunk
                for cc in range(nchunk):
                    eng.dma_start(
                        out=wsb[:, cc * kper : (cc + 1) * kper, :],
                        in_=wsrc[:, cc * kper : (cc + 1) * kper, :],
                    )
                return wsb

            def load_bias(name, dram, eng):
                # bias row D of dram -> [128, HL] tile; partition p, col m
                # holds bias value m*128 + p (halves at partition bases 0/64
                # so STT scalar operands align with their c-half)
                bc = wpool.tile([P, HL], bf16, tag=f"{name}bc")
                brow = dram[D : D + 1, :]
                eng.dma_start(
                    out=bc,
                    in_=bass.AP(
                        tensor=brow.tensor,
                        offset=brow.offset,
                        ap=[[1, P], [P, HL]],
                    ),
                )
                return bc

            # Startup-critical loads (wq, wk, cos, sin) go on the sync queue:
            # its enqueues are cheap and output DMAs don't start until
            # attention(b0) ends. wv/biases/wo ride the scalar queue.
            # cos/sin are duplicated into both partition halves so every
            # RoPE STT has all-SBUF operands on one start partition.
            wk_t = load_wT("wk", wkT, nc.sync, nchunk=8)
            wq_t = load_wT("wq", wqT, nc.sync)
            cos_sb = wpool.tile([P, T], bf16, tag="cos")
            sin_sb = wpool.tile([P, T], bf16, tag="sin")
            nc.sync.dma_start(out=cos_sb[0:hh, :], in_=cosT[:, :])
            nc.sync.dma_start(out=cos_sb[hh : 2 * hh, :], in_=cosT[:, :])
            nc.sync.dma_start(out=sin_sb[0:hh, :], in_=sinT[:, :])
            nc.sync.dma_start(out=sin_sb[hh : 2 * hh, :], in_=sinT[:, :])
            wq_bc = load_bias("wq", wqT, nc.scalar)
            wk_bc = load_bias("wk", wkT, nc.scalar)
            wv_t = load_wT("wv", wvT, nc.scalar)
            wo_t = []

            def load_wo():
                # late program position keeps wo off the startup DMA window;
                # it is first needed at attention(b0)'s out-proj drains
                for m in range(HL):
                    t = wpool.tile([P, D], bf16, tag=f"wo{m}")
                    nc.scalar.dma_start(out=t, in_=woT[m * P : (m + 1) * P, :])
                    wo_t.append(t)

            op_alt = [0]

            def emit_outproj_group(bb, tt, ds, use_psS=False):
                t0 = bb * T + tt * P
                if use_psS:
                    ps = psS.tile([P, 2 * SPAN], f32, tag="pS", name="ps_op2")[:, :SPAN]
                else:
                    ps = ps512.tile([P, SPAN], f32, tag="p512", name="ps_op")
                for m in range(HL):
                    nc.tensor.matmul(
                        ps,
                        aoT_of[bb][m][:, tt * P : (tt + 1) * P],
                        wo_t[m][:, ds * SPAN : (ds + 1) * SPAN],
                        start=(m == 0),
                        stop=(m == HL - 1),
                    )
                ob = obuf.tile([P, SPAN], bf16, tag="ob")
                op_alt[0] ^= 1
                if op_alt[0]:
                    nc.vector.tensor_copy(out=ob, in_=ps)
                else:
                    nc.scalar.activation(out=ob, in_=ps, func=Copy)
                nc.sync.dma_start(
                    out=outp[t0 : t0 + P, ds * SPAN : (ds + 1) * SPAN], in_=ob
                )

            aoT_of = {}
            op_work = []
            op_i = [0]

            def drain_op(n):
                while op_i[0] < len(op_work) and n > 0:
                    emit_outproj_group(*op_work[op_i[0]])
                    op_i[0] += 1
                    n -= 1

            for b in range(B):
                # ---- QKV projections + RoPE, processed in span pairs ----
                qT = [qkT.tile([P, T], bf16, tag=f"qT{m}", name=f"qT{m}") for m in range(HL)]
                kT = [qkT.tile([P, T], bf16, tag=f"kT{m}", name=f"kT{m}") for m in range(HL)]
                v_t = [qkv.tile([P, HL * HDH], bf16, tag=f"v{tt}", name=f"v{tt}") for tt in range(NTT)]
                ao_t = [qkv.tile([P, DOUT], bf16, tag=f"ao{tt}", name=f"ao{tt}") for tt in range(NTT)]
                aoT_of[b] = [aop.tile([P, T], bf16, tag=f"aoT{m}", name=f"aoT{m}") for m in range(HL)]

                for s2 in range(NSPAN // 2):
                    # chunk-interleave the two spans' loads so the q/k
                    # k-loop (which consumes both halves per chunk) is not
                    # gated on the second tile's tail
                    xts = [
                        xpool.tile([P, NK, SPAN], bf16, tag=f"x{half}", name=f"x{half}")
                        for half in range(2)
                    ]
                    xsrcs = [
                        xT[
                            :,
                            b * T + (2 * s2 + half) * SPAN : b * T
                            + (2 * s2 + half + 1) * SPAN,
                        ].rearrange("(ko p) t -> p ko t", p=P)
                        for half in range(2)
                    ]
                    # finer chunks for the very first pair: smaller DMAs
                    # complete sooner against the startup weight trickle, so
                    # the k-loop starts ~earlier
                    nchunk = 8 if (b == 0 and s2 == 0) else 4
                    kper = NK // nchunk
                    for cc in range(nchunk):
                        for half in range(2):
                            nc.gpsimd.dma_start(
                                out=xts[half][:, cc * kper : (cc + 1) * kper, :],
                                in_=xsrcs[half][:, cc * kper : (cc + 1) * kper, :],
                            )
                    sl2 = slice(2 * s2 * SPAN, (2 * s2 + 2) * SPAN)  # 1024 tokens

                    # q/k over the span pair: [128, 1024] psum, LDW shared.
                    # Evict psum via one scalar copy (fast bank release),
                    # then RoPE on the vector engine.  k runs FIRST so its
                    # RoPE chain (which gates the next batch's S.T) hides
                    # under the q matmuls.
                    for dst, wsb, bc in ((kT, wk_t, wk_bc), (qT, wq_t, wq_bc)):
                        for m in range(HL):
                            ps = psS.tile([P, 2 * SPAN], f32, tag="pS", name="ps_qk")
                            for k in range(NK):
                                for half in range(2):
                                    nc.tensor.matmul(
                                        ps[:, half * SPAN : (half + 1) * SPAN],
                                        wsb[:, k, m * P : (m + 1) * P],
                                        xts[half][:, k, :],
                                        start=(k == 0),
                                        stop=(k == NK - 1),
                                    )
                            c = cpool.tile([P, 2 * SPAN], bf16, tag="c", name="c_qk")
                            nc.scalar.activation(out=c, in_=ps, func=Copy)
                            lo = slice(0, hh)
                            hi = slice(hh, 2 * hh)
                            bm = bc[:, m : m + 1]
                            # u1 = (c+b)*cos, u2 = (c+b)*sin (full width, all
                            # operands at start partition 0 for the verifier);
                            # sw = u2 with halves swapped (zero-add TTs: the
                            # verifier allows TT output on a different start
                            # partition than its — aligned — inputs)
                            u1 = misc.tile([P, 2 * SPAN], bf16, tag="u1", name="u1")
                            u2 = misc.tile([P, 2 * SPAN], bf16, tag="u2", name="u2")
                            sw = misc.tile([P, 2 * SPAN], bf16, tag="sw", name="sw")
                            nc.vector.scalar_tensor_tensor(
                                u1, c, bm, cos_sb[:, sl2], add, mult
                            )
                            nc.vector.scalar_tensor_tensor(
                                u2, c, bm, sin_sb[:, sl2], add, mult
                            )
                            nc.vector.tensor_add(sw[lo, :], u2[hi, :], zeroT[hi, :])
                            nc.vector.tensor_add(sw[hi, :], u2[lo, :], zeroT[lo, :])
                            nc.vector.tensor_sub(dst[m][lo, sl2], u1[lo, :], sw[lo, :])
                            nc.vector.tensor_add(dst[m][hi, sl2], u1[hi, :], sw[hi, :])

                    # v: per 128-token tile; evict psum -> head slots with a
                    # single 3D scalar copy (v bias is handled on the host)
                    for half in range(2):
                        s = 2 * s2 + half
                        for tt in range(SPAN // P):
                            gt = s * (SPAN // P) + tt
                            sl_p = slice(tt * P, (tt + 1) * P)
                            ps = ps512.tile([P, SPAN], f32, tag="p512", name="ps_v")
                            psv = ps[:, :DOUT]
                            for k in range(NK):
                                nc.tensor.matmul(
                                    psv,
                                    xts[half][:, k, sl_p],
                                    wv_t[:, k, :],
                                    start=(k == 0),
                                    stop=(k == NK - 1),
                                )
                            vt = v_t[gt]
                            ones_ap = bass.AP(
                                tensor=vt.tensor,
                                offset=vt.offset + DH,
                                ap=[vt.ap[0], [HDH, HL]],
                            )
                            nc.vector.memset(ones_ap, 1.0)
                            vdst = bass.AP(
                                tensor=vt.tensor,
                                offset=vt.offset,
                                ap=[vt.ap[0], [HDH, HL], [1, DH]],
                            )
                            vsrc = bass.AP(
                                tensor=psv.tensor,
                                offset=psv.offset,
                                ap=[psv.ap[0], [DH, HL], [1, DH]],
                            )
                            nc.scalar.activation(out=vdst, in_=vsrc, func=Copy)

                if b == 0:
                    load_wo()

                # ---- attention (b): s outer, m inner; transposes + out-proj
                # enqueue per span so OP(b) overlaps attention(b) itself ----
                for s in range(NSPAN):
                    sl_q = slice(s * SPAN, (s + 1) * SPAN)
                    for m in range(HL):
                        # S.T pairs: two k-tiles per [P, 1024] psum -> one
                        # exp.  The S.T loop is exp-paced (~0.92us/block vs
                        # 0.43us of S.T matmuls), so drip one out-proj group
                        # into EVERY block to keep the PE fed.
                        etiles = []
                        for kt2 in range(NTT // 2):
                            ps = psS.tile([P, 2 * SPAN], f32, tag="pS", name="ps_s")
                            for half in range(2):
                                nc.tensor.matmul(
                                    ps[:, half * SPAN : (half + 1) * SPAN],
                                    kT[m][:, (2 * kt2 + half) * P : (2 * kt2 + half + 1) * P],
                                    qT[m][:, sl_q],
                                    start=True,
                                    stop=True,
                                )
                            e = epool.tile([P, 2 * SPAN], bf16, tag=f"e{kt2}")
                            nc.scalar.activation(
                                out=e, in_=ps, func=mybir.ActivationFunctionType.Exp
                            )
                            etiles.append(e[:, 0:SPAN])
                            etiles.append(e[:, SPAN : 2 * SPAN])
                            drain_op(1)
                        last_span = b == B - 1 and s == NSPAN - 1
                        for tt in range(SPAN // P):
                            gt = s * (SPAN // P) + tt
                            sl_p = slice(tt * P, (tt + 1) * P)
                            po = pspv.tile([P, DH + 1], f32, tag="pv", name="po")
                            for kt in range(NTT):
                                nc.tensor.matmul(
                                    po,
                                    etiles[kt][:, sl_p],
                                    v_t[kt][:, m * HDH : (m + 1) * HDH],
                                    start=(kt == 0),
                                    stop=(kt == NTT - 1),
                                )
                            rec = recp.tile([P, 1], f32, tag="rec")
                            nc.vector.reciprocal(rec, po[:, DH : DH + 1])
                            nc.vector.tensor_scalar_mul(
                                ao_t[gt][:, m * DH : (m + 1) * DH],
                                po[:, 0:DH],
                                rec,
                            )
                            if last_span and m == HL - 1:
                                # final span: transpose + out-proj per tile,
                                # alternating psum pools (psS is idle here)
                                # so the tail pipelines instead of trickling
                                for mm in range(HL):
                                    pt = pspv.tile([P, P], bf16, tag="pv", name="pt")
                                    nc.tensor.transpose(
                                        pt, ao_t[gt][:, mm * DH : (mm + 1) * DH], ident
                                    )
                                    nc.vector.tensor_copy(
                                        out=aoT_of[b][mm][:, gt * P : (gt + 1) * P],
                                        in_=pt,
                                    )
                                for ds in range(D // SPAN):
                                    emit_outproj_group(b, gt, ds, use_psS=bool(ds % 2))

                    if not (b == B - 1 and s == NSPAN - 1):
                        # transpose this span's attn_out tiles: [t, d] -> [d, t]
                        for tt in range(SPAN // P):
                            gt = s * (SPAN // P) + tt
                            for m in range(HL):
                                pt = pspv.tile([P, P], bf16, tag="pv", name="pt")
                                nc.tensor.transpose(
                                    pt, ao_t[gt][:, m * DH : (m + 1) * DH], ident
                                )
                                nc.vector.tensor_copy(
                                    out=aoT_of[b][m][:, gt * P : (gt + 1) * P],
                                    in_=pt,
                                )
                        for ds in range(D // SPAN):
                            for tt in range(SPAN // P):
                                op_work.append((b, s * (SPAN // P) + tt, ds))

            # tail: drain whatever out-proj work remains
            drain_op(len(op_work))
    return nc


_nc_cache = None


def _get_nc():
    global _nc_cache
    if _nc_cache is None:
        _nc_cache = _build_nc()
    return _nc_cache


# ---------------------------------------------------------------------------
# Host wrapper
# ---------------------------------------------------------------------------


def _prep_inputs(x, pos, wq_w, wq_b, wk_w, wk_b, wv_w, wv_b, wo_w, wo_b):
    x2 = np.asarray(x, np.float32).reshape(BT, D)
    xT = np.ascontiguousarray(x2.T).astype(BF16)

    pos1 = np.asarray(pos, np.float32).reshape(T)
    freq = (1.0 / 10000.0 ** (np.arange(0, DH, 2, np.float32) / DH)).astype(np.float32)
    ang = pos1[None, :] * freq[:, None]  # [64, T]
    cosT = np.cos(ang).astype(BF16)
    sinT = np.sin(ang).astype(BF16)

    scale = np.float32(1.0 / np.sqrt(DH))

    def wslice(w, bvec, c, s=None, with_bias=True):
        w = np.asarray(w, np.float32)
        ws = w[c * DOUT : (c + 1) * DOUT]  # [256, D]
        if s is not None:
            ws = ws * s
        if not with_bias:
            return np.ascontiguousarray(ws.T).astype(BF16)
        bvec = np.asarray(bvec, np.float32)
        bs = bvec[c * DOUT : (c + 1) * DOUT]
        if s is not None:
            bs = bs * s
        out = np.empty((D + 1, DOUT), BF16)
        out[:D] = ws.T.astype(BF16)
        out[D] = bs.astype(BF16)
        return out

    in_maps = []
    for c in range(N_CORES):
        woTc = (
            np.asarray(wo_w, np.float32)[:, c * DOUT : (c + 1) * DOUT]
            .T.astype(BF16)
            .copy()
        )
        in_maps.append(
            {
                "xT": xT,
                "wqT": wslice(wq_w, wq_b, c, scale),
                "wkT": wslice(wk_w, wk_b, c),
                "wvT": wslice(wv_w, None, c, with_bias=False),
                "woT": woTc,
                "cosT": cosT,
                "sinT": sinT,
            }
        )
    return in_maps


def _run(in_maps, trace=False):
    _install_compile_patch()
    from concourse.bass_utils import run_bass_kernel_spmd

    nc = _get_nc()
    return run_bass_kernel_spmd(
        nc, in_maps, core_ids=list(range(N_CORES)), trace=trace
    )


def kernel(**inputs):
    inputs = {k: np.asarray(v) for k, v in inputs.items()}
    in_maps = _prep_inputs(**inputs)
    r = _run(in_maps, trace=False)
    acc = np.zeros((BT, D), np.float64)
    for c in range(N_CORES):
        acc += r.results[c]["out"].astype(np.float64)
    # v bias flows through the output projection as a constant row:
    # P @ (V + 1 b^T) / d = P @ V / d + b^T  (softmax rows sum to 1)
    wv_b = np.asarray(inputs["wv_b"], np.float32)
    wo_w = np.asarray(inputs["wo_w"], np.float32)
    wo_b = np.asarray(inputs["wo_b"], np.float32)
    acc += (wv_b @ wo_w.T + wo_b).astype(np.float64)
    return acc.astype(np.float32).reshape(B, T, D)
